# revision 96
# baseline (speedup 1.0000x reference)
"""Trainium2 Bass kernel for sliding-window (±64) multi-head attention.

Reference computation (seq=4096, hidden=768, 12 heads x 64, RoPE, window 128):
    qkv = qkv_weight @ x ; q,k = rope(q,k) ; scores = q^T k / 8 + band_mask
    attn = softmax(scores) @ v ; out = out_weight @ attn

Sharding: sequence-parallel over 8 cores. Core c owns queries
[512c, 512c+512) and computes K/V over the haloed span [512c-64, 512c+576)
(zero-padded at the sequence edges; padding is killed by the band mask).
Each core runs an identical Bass program on different data; the full output
is reassembled on host by concatenation (no collectives needed).

Key structural choices (all bf16 matmuls; fp8 fails the 2e-2 rel-err gate):
- The HWDGE descriptor generator is one shared unit (~630ns per DMA) and
  the DMA wire is one shared serial FIFO, so inputs are packed into few
  large host-side tensors (wq|wk per head pair; cos|sin|perms|mask|diag|sel
  in one misc tensor) and DMA'd on the SP queue in strict first-use
  priority order; mid-run weights trickle between the P transposes.
- A PE p-state warm-up (throwaway matmuls on a zeroed tile) keeps the
  tensor engine busy while the first operands stream in, so the real
  projections start at the full 2.4GHz p-state.
- RoPE without evacuating the projection PSUM: since cos/sin are 32-periodic
  across each head's rotation pairs, rot(q*sin) == rot(q)*sin, so
  q_rope = q*cos + PERMS^T @ (q*sin).
- Softmax WITHOUT per-unit normalization on the critical path: the DVE
  scalar_tensor_tensor applies the multiplicative band mask and emits the
  row sums (accum_out) as a side effect; the un-normalized P goes straight
  to the transpose DMA and PV matmul. Normalization happens once per head
  pair at PSUM evacuation: row sums [128q, 8slots] -> bf16 -> PE transpose
  (identity-rhs matmul) -> bf16 reciprocal -> tiny selector matmuls
  broadcast the per-(h,q) reciprocals into a [128, 512] PSUM tile -> ACT
  copy to SBUF -> the evacuation is a single DVE tensor_tensor multiply
  (same cost as the copy it replaces; engines may read only one PSUM
  operand per instruction).
- P^T for the PV matmul comes from the DMA xbar transpose engine
  (dma_start_transpose): per unit one DMA does all four 128x128 block
  transposes (written packed at pitch 128 into a padded tile). The
  8-stage modulo pipeline gives the transpose ~3 steps of latency slack.
- Output projection: 5-chunk partials are computed while the last head
  pair's attention drains (oc 0-2 partials in f32, combined with the tail
  chunk on DVE; oc 3-5 in bf16, added back on PE via an identity matmul
  and evacuated on ACT, overlapping the DVE combines); the result ships
  in 2-chunk DMAs ordered by combine completion.
- Band mask is 3 shared [128,256] slots (first block / interior / last
  block); the output DMA is bf16 and upcast on host.
"""

import os
import sys

import numpy as np

for _p in ("/opt/trn_rl_repo",):
    if _p not in sys.path and os.path.isdir(_p):
        sys.path.insert(0, _p)

import ml_dtypes

import concourse.bass as bass
import concourse.bacc as bacc
import concourse.tile as tile
from concourse import mybir
from concourse.bass_utils import run_bass_kernel_spmd

F32 = mybir.dt.float32
BF16 = mybir.dt.bfloat16

N_CORES = 8
SEQ = 4096
S_CORE = SEQ // N_CORES  # 512 queries per core
HALO = 64                # window // 2
SPAN = S_CORE + 2 * HALO  # 640 keys per core
HID = 768
NH = 12
DH = 64
NCH = HID // 128         # 6 contraction chunks
NHP = NH // 2            # 6 head pairs
NQB = S_CORE // 128      # 4 query blocks per core
NSC = SPAN // 128        # 5 key chunks per core
KSPAN = 256              # key span per query block
PTS_PITCH = 136          # padded block pitch of the transposed-P tile
WV_HALF = HID // 2       # 384

_BUILD_CACHE = {}


def _build(add_mask: bool, reps: int = 1):
    """Build + compile the per-core Bass program (shared by all 8 cores)."""
    nc = bacc.Bacc("TRN2", target_bir_lowering=False, debug=False, num_devices=N_CORES)

    xin_d = nc.dram_tensor("xin", [128, NCH * SPAN], BF16, kind="ExternalInput")
    # q|k weights packed per head pair: [hp, (q 768 | k 768)]
    wqkt_d = nc.dram_tensor(
        "wqkt", [128, NHP * 2 * NCH * 128], BF16, kind="ExternalInput"
    )
    wvt_d = nc.dram_tensor("wvt", [128, NCH * HID], BF16, kind="ExternalInput")
    wot_d = nc.dram_tensor("wot", [128, NCH * HID], BF16, kind="ExternalInput")
    # misc pack: cos 640 | sin 640 | perms 128 | mb 768 | diag 128 | sel 512
    MISC_W = SPAN * 2 + 128 + 3 * KSPAN + 128 + NQB * 128
    misc_d = nc.dram_tensor("miscb", [128, MISC_W], BF16, kind="ExternalInput")
    if add_mask:
        maskf_d = nc.dram_tensor(
            "maskf", [128, NQB * 2 * KSPAN], F32, kind="ExternalInput"
        )
    out_d = nc.dram_tensor("out", [128, NCH * S_CORE], BF16, kind="ExternalOutput")

    mult = mybir.AluOpType.mult
    addop = mybir.AluOpType.add
    exp = mybir.ActivationFunctionType.Exp

    with tile.TileContext(nc) as tc:
        from contextlib import ExitStack

        for _rep in range(reps):
          with ExitStack() as ctx:
            const = ctx.enter_context(tc.tile_pool(name="const", bufs=1))
            sb = ctx.enter_context(tc.tile_pool(name="sb", bufs=1))
            rope_p = ctx.enter_context(tc.tile_pool(name="rope", bufs=10))
            attnp = ctx.enter_context(tc.tile_pool(name="attnp", bufs=12))
            scal = ctx.enter_context(tc.tile_pool(name="scal", bufs=12))
            outp = ctx.enter_context(tc.tile_pool(name="outp", bufs=6))
            ps_proj = ctx.enter_context(
                tc.tile_pool(name="ps_proj", bufs=3, space="PSUM")
            )
            ps_att = ctx.enter_context(
                tc.tile_pool(name="ps_att", bufs=3, space="PSUM")
            )
            ps_o = ctx.enter_context(tc.tile_pool(name="ps_o", bufs=2, space="PSUM"))

            # ---- input tiles ----
            XIN = const.tile([128, NCH * SPAN], BF16, tag="XIN")
            WQKT = const.tile([128, NHP * 2 * NCH * 128], BF16, tag="WQKT")
            WVT = const.tile([128, NCH * HID], BF16, tag="WVT")  # (hf, k, 384)
            MISC = const.tile([128, MISC_W], BF16, tag="MISC")
            COS = MISC[:, 0:SPAN]
            SIN = MISC[:, SPAN : 2 * SPAN]
            PERMS = MISC[:, 2 * SPAN : 2 * SPAN + 128]
            _mb0 = 2 * SPAN + 128
            MB = MISC[:, _mb0 : _mb0 + 3 * KSPAN]
            _dg0 = _mb0 + 3 * KSPAN
            DIAG = MISC[:, _dg0 : _dg0 + 128]
            _sl0 = _dg0 + 128
            SELB = MISC[0:8, _sl0 : _sl0 + NQB * 128]
            WOT = sb.tile([128, NCH * HID], BF16, tag="WOT")

            def dma_xin(k, eng):
                eng.dma_start(
                    out=XIN[:, k * SPAN : (k + 1) * SPAN],
                    in_=xin_d.ap()[:, k * SPAN : (k + 1) * SPAN],
                )

            def dma_wqk(hp, eng, half=None):
                w = 2 * NCH * 128
                lo, hi = hp * w, (hp + 1) * w
                if half == 0:
                    hi = lo + w // 2
                elif half == 1:
                    lo = lo + w // 2
                eng.dma_start(out=WQKT[:, lo:hi], in_=wqkt_d.ap()[:, lo:hi])

            def dma_wvt_half(hf, eng):
                w = NCH * WV_HALF
                eng.dma_start(
                    out=WVT[:, hf * w : (hf + 1) * w],
                    in_=wvt_d.ap()[:, hf * w : (hf + 1) * w],
                )

            # ---- startup DMA schedule. The HWDGE descriptor generator is a
            # single shared unit (~630ns per DMA) and the DMA wire is one
            # shared serial FIFO, so everything goes on the SP queue in
            # strict priority order of first use. Later bulk weights are
            # emitted inside the pipeline (extra schedule); since the SP SEQ
            # is held through each P-transpose's data wait, they naturally
            # trickle in without stealing early wire bandwidth.
            def dma_xin3(lo, eng):
                eng.dma_start(
                    out=XIN[:, lo * SPAN : (lo + 3) * SPAN],
                    in_=xin_d.ap()[:, lo * SPAN : (lo + 3) * SPAN],
                )

            # PE p-state warm-up: throwaway matmuls on a zeroed tile keep
            # the tensor engine continuously busy from t~0.2us until the
            # first real operands land (~4.3us), so the projection matmuls
            # start at the full 2.4GHz p-state instead of mid-ramp.
            WARM = sb.tile([128, 512], BF16, tag="WARM")
            nc.gpsimd.memset(WARM[:], 0.0)
            wps = ps_o.tile([128, 512], F32, tag="o", name="warm")
            for i in range(12):
                nc.tensor.matmul(wps[:], WARM[:, 0:128], WARM[:],
                                 start=(i == 0), stop=(i == 11))

            dma_wqk(0, nc.sync, half=0)   # wq of hp0
            dma_xin3(0, nc.sync)          # x chunks 0-2 (q proj can start)
            dma_xin3(3, nc.sync)          # x chunks 3-5
            dma_wqk(0, nc.sync, half=1)   # wk of hp0
            nc.sync.dma_start(   # cos | sin | perms
                out=MISC[:, 0 : 2 * SPAN + 128],
                in_=misc_d.ap()[:, 0 : 2 * SPAN + 128],
            )
            dma_wvt_half(0, nc.sync)
            dma_wqk(1, nc.sync)
            nc.sync.dma_start(   # mb | diag | sel
                out=MISC[:, 2 * SPAN + 128 : MISC_W],
                in_=misc_d.ap()[:, 2 * SPAN + 128 : MISC_W],
            )
            if add_mask:
                MF = const.tile([128, NQB * 2 * KSPAN], F32, tag="MF")
                nc.sync.dma_start(out=MF[:], in_=maskf_d.ap())

            # persistent intermediates
            Qs = sb.tile([128, NHP * S_CORE], BF16, tag="Qs")   # [2hd, (hp, s)]
            Ks = sb.tile([128, NHP * SPAN], BF16, tag="Ks")     # [2hd, (hp, s)]
            VT = sb.tile([128, NSC * HID], BF16, tag="VT")      # [s, (chunk, hd)]
            AT = sb.tile([128, NCH * S_CORE], BF16, tag="AT")   # [c, (cchunk, s)]

            def xs(k, lo, w):
                return XIN[:, k * SPAN + lo : k * SPAN + lo + w]

            # ---- V^T projection: VT[s, hd] per 128-key chunk (bf16).
            # Split by output half: half 0 feeds heads 0-5 (head pairs 0-2,
            # consumed from step 5), half 1 feeds heads 6-11 (from step 17),
            # so the second V-weight half can stream in late.
            def vt_unit(sc, hf):
                w = WV_HALF  # 384
                vp = ps_proj.tile([128, w], F32, tag="proj")
                for k in range(NCH):
                    nc.tensor.matmul(
                        vp[:],
                        xs(k, sc * 128, 128),
                        WVT[:, hf * NCH * w + k * w : hf * NCH * w + (k + 1) * w],
                        start=(k == 0),
                        stop=(k == NCH - 1),
                    )
                nc.scalar.copy(
                    VT[:, sc * HID + hf * w : sc * HID + (hf + 1) * w], vp[:]
                )

            # ---- per head pair: project Q,K then rope, in three phases so
            # the in-order PE queue never waits on elementwise results.
            rope_st = {}

            def proj_mm(hp):
                w2 = 2 * NCH * 128
                wq = WQKT[:, hp * w2 : hp * w2 + NCH * 128]
                wk = WQKT[:, hp * w2 + NCH * 128 : (hp + 1) * w2]
                blocks = []
                qp = ps_proj.tile([128, S_CORE], F32, tag="proj")
                for k in range(NCH):
                    nc.tensor.matmul(
                        qp[:],
                        wq[:, k * 128 : (k + 1) * 128],
                        xs(k, HALO, S_CORE),
                        start=(k == 0),
                        stop=(k == NCH - 1),
                    )
                blocks.append((qp, HALO, S_CORE,
                               Qs[:, hp * S_CORE : (hp + 1) * S_CORE]))
                for half in range(2):
                    w = SPAN // 2  # 320
                    kp = ps_proj.tile([128, w], F32, tag="proj")
                    for k in range(NCH):
                        nc.tensor.matmul(
                            kp[:],
                            wk[:, k * 128 : (k + 1) * 128],
                            xs(k, half * w, w),
                            start=(k == 0),
                            stop=(k == NCH - 1),
                        )
                    blocks.append(
                        (kp, half * w, w,
                         Ks[:, hp * SPAN + half * w : hp * SPAN + (half + 1) * w])
                    )
                rope_st[hp] = blocks

            def rope_mults(hp):
                out = []
                for i, (p, lo, w, dst) in enumerate(rope_st[hp]):
                    qsb = rope_p.tile([128, S_CORE], BF16, tag="qsb")
                    nc.scalar.copy(qsb[:, :w], p[:])
                    m1 = rope_p.tile([128, S_CORE], BF16, tag="m1")
                    m2 = rope_p.tile([128, S_CORE], BF16, tag="m2")
                    nc.gpsimd.tensor_tensor(
                        m1[:, :w], qsb[:, :w], COS[:, lo : lo + w], op=mult
                    )
                    nc.vector.tensor_tensor(
                        m2[:, :w], qsb[:, :w], SIN[:, lo : lo + w], op=mult
                    )
                    out.append((m1, m2, w, dst))
                rope_st[hp] = out

            def rope_rot(hp):
                for i, (m1, m2, w, dst) in enumerate(rope_st[hp]):
                    rot = ps_proj.tile([128, S_CORE], F32, tag="proj")
                    nc.tensor.matmul(
                        rot[:, :w], PERMS[:], m2[:, :w], start=True, stop=True
                    )
                    nc.vector.tensor_tensor(dst, m1[:, :w], rot[:, :w], op=addop)
                del rope_st[hp]

            # ---- attention: modulo software pipeline over the 24
            # (head-pair, query-block) units ----
            sss = {}   # hp -> [128, 8] f32 row-sum tile
            rrs = {}   # hp -> [8, 128] bf16 reciprocal tile
            o2s = {}

            def stage_scores(st):
                hp, qb = st["hp"], st["qb"]
                if qb == 0:
                    sss[hp] = scal.tile([128, 8], F32, tag="ss",
                                        name=f"ss_{hp}")
                ss = []
                for h in range(2):
                    s1 = ps_att.tile([128, 2 * KSPAN], F32, tag="att",
                                     name=f"s_{hp}_{qb}_{h}")
                    nc.tensor.matmul(
                        s1[:, :KSPAN],
                        Qs[64 * h : 64 * (h + 1),
                           hp * S_CORE + qb * 128 : hp * S_CORE + (qb + 1) * 128],
                        Ks[64 * h : 64 * (h + 1),
                           hp * SPAN + qb * 128 : hp * SPAN + qb * 128 + KSPAN],
                        start=True,
                        stop=True,
                    )
                    ss.append(s1)
                st["sc"] = ss

            def stage_exp(st):
                praw = attnp.tile([128, 2 * KSPAN], BF16, tag="praw")
                moff = st["qb"] * 2 * KSPAN
                for h in range(2):
                    sh = st["sc"][h][:, :KSPAN]
                    if add_mask:
                        nc.vector.tensor_tensor(
                            sh, sh,
                            MF[:, moff + h * KSPAN : moff + (h + 1) * KSPAN],
                            op=addop,
                        )
                    nc.scalar.activation(
                        praw[:, h * KSPAN : (h + 1) * KSPAN], sh, exp
                    )
                st["praw"] = praw
                del st["sc"]

            def stage_dve(st):
                # multiplicative band mask + row-sum side channel; the
                # un-normalized P goes straight to the transpose.
                hp, qb = st["hp"], st["qb"]
                slot = 0 if qb == 0 else (2 if qb == NQB - 1 else 1)
                praw = st["praw"]
                P = attnp.tile([128, 2 * KSPAN], BF16, tag="P")
                ss = sss[hp]
                for h in range(2):
                    nc.vector.scalar_tensor_tensor(
                        out=P[:, h * KSPAN : (h + 1) * KSPAN],
                        in0=praw[:, h * KSPAN : (h + 1) * KSPAN],
                        scalar=1.0,
                        in1=MB[:, slot * KSPAN : (slot + 1) * KSPAN],
                        op0=mult,
                        op1=mult,
                        accum_out=ss[:, 2 * qb + h : 2 * qb + h + 1],
                    )
                st["P"] = P
                del st["praw"]

            def stage_pt(st):
                # all four 128x128 block transposes in one xbar DMA; the HW
                # xbar writes the transposed blocks PACKED at pitch 128 into
                # the padded-pitch tile.
                hp, qb = st["hp"], st["qb"]
                P = st["P"]
                ptsp = attnp.tile([128, 4 * PTS_PITCH], BF16, tag="pts")
                pts3 = ptsp[:].rearrange("p (b j) -> p b j", b=4)
                nc.sync.dma_start_transpose(out=pts3[:, :, 0:128], in_=P[:])
                st["pts"] = ptsp
                del st["P"]
                if qb == NQB - 1:
                    # start the per-head-pair reciprocal chain: row sums
                    # f32 -> bf16 (Pool, off the critical engines)
                    ssb = scal.tile([128, 8], BF16, tag="ssb")
                    nc.gpsimd.tensor_copy(ssb[:], sss[hp][:])
                    st["ssb"] = ssb

            def stage_bubble(st):
                # spacing stage: gives the pt transpose DMA time to land
                # before pv consumes it. For qb3 also finish the reciprocal
                # chain: PE transpose (identity-rhs matmul) -> reciprocal
                # -> bf16.
                hp, qb = st["hp"], st["qb"]
                if qb == NQB - 1:
                    sst = ps_proj.tile([8, 128], F32, tag="proj")
                    nc.tensor.matmul(
                        sst[:], st["ssb"][:], DIAG, start=True, stop=True
                    )
                    rr = scal.tile([8, 128], BF16, tag="rrb")
                    with nc.allow_low_precision("bf16 softmax scale is inside the rel-err budget"):
                        nc.vector.reciprocal(rr[:], sst[:])
                    rrs[hp] = rr
                    del sss[hp], st["ssb"]

            def stage_pv(st):
                hp, qb = st["hp"], st["qb"]
                if qb == 0:
                    o2s[hp] = ps_o.tile([128, S_CORE], F32, tag="o",
                                        name=f"o2_{hp}")
                o2 = o2s[hp]
                pts = st["pts"]
                for h in range(2):
                    hg = hp * 2 + h
                    osl = o2[64 * h : 64 * (h + 1), qb * 128 : (qb + 1) * 128]
                    tp = (0, 64 * h)
                    nc.tensor.matmul(
                        osl,
                        VT[:, qb * HID + hg * 64 : qb * HID + hg * 64 + 64],
                        pts[:, (2 * h) * 128 : (2 * h + 1) * 128],
                        start=True, stop=False, tile_position=tp,
                    )
                    nc.tensor.matmul(
                        osl,
                        VT[:, (qb + 1) * HID + hg * 64 : (qb + 1) * HID + hg * 64 + 64],
                        pts[:, (2 * h + 1) * 128 : (2 * h + 2) * 128],
                        start=False, stop=True, tile_position=tp,
                    )
                del st["pts"]
                if qb == NQB - 1:
                    nc.vector.tensor_tensor(
                        AT[:, hp * S_CORE : (hp + 1) * S_CORE], o2[:],
                        st["bcs"][:], op=mult,
                    )
                    del o2s[hp], rrs[hp], st["bcs"]

            # 5-chunk out-projection partials: oc 0-2 kept in f32 (combined
            # on DVE at the tail), oc 3-5 in bf16 (added back on PE via an
            # identity matmul, evacuated on ACT — overlaps the DVE combines)
            PO1 = sb.tile([128, 3 * S_CORE], F32, tag="PO1")
            PO1B = sb.tile([128, 3 * S_CORE], BF16, tag="PO1B")

            def outproj_part1(ocs):
                for oc in ocs:
                    ops = ps_proj.tile([128, S_CORE], F32, tag="proj")
                    for k in range(5):
                        nc.tensor.matmul(
                            ops[:],
                            WOT[:, k * HID + oc * 128 : k * HID + (oc + 1) * 128],
                            AT[:, k * S_CORE : (k + 1) * S_CORE],
                            start=(k == 0),
                            stop=(k == 4),
                        )
                    if oc in (1, 3, 5):
                        nc.scalar.copy(
                            PO1B[:, (oc // 2) * S_CORE : (oc // 2 + 1) * S_CORE],
                            ops[:],
                        )
                    else:
                        nc.scalar.copy(
                            PO1[:, (oc // 2) * S_CORE : (oc // 2 + 1) * S_CORE],
                            ops[:],
                        )

            proj_mm(0)
            rope_mults(0)
            vt_unit(0, 0)
            rope_rot(0)
            vt_unit(1, 0)

            units = [
                {"hp": hp, "qb": qb} for hp in range(NHP) for qb in range(NQB)
            ]
            def stage_bubble2(st):
                hp, qb = st["hp"], st["qb"]
                if qb == NQB - 1:
                    # broadcast the per-(h,q) reciprocals into PSUM via
                    # selector matmuls and copy to SBUF on ACT one step
                    # before the pv that consumes them (the evacuation may
                    # read only one PSUM operand).
                    bc = ps_o.tile([128, S_CORE], F32, tag="o",
                                   name=f"bc_{hp}")
                    rr = rrs[hp]
                    for qq in range(NQB):
                        nc.tensor.matmul(
                            bc[:, qq * 128 : (qq + 1) * 128],
                            SELB[:, qq * 128 : (qq + 1) * 128],
                            rr[:],
                            start=True, stop=True,
                        )
                    bcs = attnp.tile([128, S_CORE], BF16, tag="bcs")
                    nc.scalar.copy(bcs[:], bc[:])
                    st["bcs"] = bcs

            def stage_bubble3(st):
                pass

            stages = [stage_scores, stage_exp, stage_dve, stage_pt,
                      stage_bubble, stage_bubble2, stage_bubble3, stage_pv]
            NU = len(units)
            ND = len(stages)
            extra = {
                0: [lambda: proj_mm(1)],
                1: [lambda: rope_mults(1)],
                2: [lambda: rope_rot(1)],
                3: [lambda: vt_unit(2, 0)],
                4: [lambda: vt_unit(3, 0)],
                5: [lambda: vt_unit(4, 0)],
                8: [lambda: vt_unit(0, 1)],
                9: [lambda: vt_unit(1, 1)],
                10: [lambda: vt_unit(2, 1)],
                11: [lambda: vt_unit(3, 1)],
                12: [lambda: vt_unit(4, 1)],
            }
            for hp_ in range(2, NHP):
                # one extra step of lead vs the attention units so the rope
                # chain's cross-engine latency stays off the critical path
                base = 4 * (hp_ - 1)
                extra.setdefault(base + 0, []).append(
                    lambda h=hp_: proj_mm(h))
                extra.setdefault(base + 1, []).append(
                    lambda h=hp_: rope_mults(h))
                extra.setdefault(base + 2, []).append(
                    lambda h=hp_: rope_rot(h))
                # prefetch the NEXT head pair's q/k weights ahead on the SP
                # queue (trickles between P transposes)
                extra.setdefault(max(0, base - 4), []).append(
                    lambda h=hp_: dma_wqk(h, nc.sync))
            # second V-weight half (consumed from step 8) and out-projection
            # weights (consumed from step 24) trickle on SP, split into
            # pieces so no single DMA head-of-line-blocks the P transposes
            w_v = NCH * WV_HALF
            for i, st_ in enumerate((5, 6)):
                extra.setdefault(st_, []).append(
                    lambda i=i: nc.sync.dma_start(
                        out=WVT[:, w_v + i * w_v // 2 : w_v + (i + 1) * w_v // 2],
                        in_=wvt_d.ap()[:, w_v + i * w_v // 2 : w_v + (i + 1) * w_v // 2],
                    ))
            w_o = NCH * HID // 4
            for i, st_ in enumerate((6, 7, 8, 9)):
                extra.setdefault(st_, []).append(
                    lambda i=i: nc.sync.dma_start(
                        out=WOT[:, i * w_o : (i + 1) * w_o],
                        in_=wot_d.ap()[:, i * w_o : (i + 1) * w_o],
                    ))

            # out-projection part1 interleaves with the last head pair's
            # attention units. AT chunk k is written by pv(hp=k, qb3) at
            # step 4k+3+(ND-1); chunk 4 lands at step 25, so part1 (which
            # contracts chunks 0-4) may start at 25.
            part1_sched = {26: (0, 1), 27: (2,), 28: (3,), 29: (4,), 30: (5,)}

            for step in range(NU + ND - 1):
                for k in range(ND):
                    idx = step - k
                    if 0 <= idx < NU:
                        stages[k](units[idx])
                for fn in extra.get(step, ()):
                    fn()
                if step in part1_sched:
                    outproj_part1(part1_sched[step])

            # ---- output projection tail: last contraction chunk. oc 0-2
            # combine PSUM + PO1 on DVE; oc 3-5 add the bf16 partial back on
            # PE (identity-stationary matmul) and evacuate on ACT. Combines
            # write into one contiguous tile so each 3-chunk group ships in
            # a single DMA (HWDGE generation is serialized, ~630ns per DMA).
            OT = outp.tile([128, NCH * S_CORE], BF16, tag="ot")
            # each shipped pair gets one DVE combine (even oc) and one ACT
            # combine (odd oc) so the two engines drain the tail in parallel
            for oc in (1, 0, 3, 2, 5, 4):
                hold = oc in (1, 3, 5)
                pool = ps_att if hold else ps_proj
                ops = pool.tile([128, S_CORE], F32,
                                tag="att" if hold else "proj")
                nc.tensor.matmul(
                    ops[:],
                    WOT[:, 5 * HID + oc * 128 : 5 * HID + (oc + 1) * 128],
                    AT[:, 5 * S_CORE : 6 * S_CORE],
                    start=True,
                    stop=False if hold else True,
                )
                ot = OT[:, oc * S_CORE : (oc + 1) * S_CORE]
                if hold:
                    nc.tensor.matmul(
                        ops[:], DIAG,
                        PO1B[:, (oc // 2) * S_CORE : (oc // 2 + 1) * S_CORE],
                        start=False, stop=True,
                    )
                    nc.scalar.copy(ot, ops[:])
                else:
                    nc.vector.scalar_tensor_tensor(
                        out=ot, in0=ops[:], scalar=1.0,
                        in1=PO1[:, (oc // 2) * S_CORE : (oc // 2 + 1) * S_CORE],
                        op0=mult, op1=addop,
                    )
            # ship in 2-chunk pieces ordered by combine completion
            for pair, eng in (((0, 1), nc.sync), ((2, 3), nc.scalar),
                              ((4, 5), nc.sync)):
                lo = pair[0] * S_CORE
                hi = (pair[1] + 1) * S_CORE
                eng.dma_start(out=out_d.ap()[:, lo:hi], in_=OT[:, lo:hi])

    nc.compile()
    return nc


def get_program(add_mask: bool, reps: int = 1):
    key = (add_mask, reps)
    if key not in _BUILD_CACHE:
        _BUILD_CACHE[key] = _build(add_mask, reps)
    return _BUILD_CACHE[key]


def _pack_chunked(a, nch, w):
    """[nch*128, w] row-major -> [128, nch*w] with chunk-major free dim."""
    return np.ascontiguousarray(
        a.reshape(nch, 128, w).transpose(1, 0, 2).reshape(128, nch * w)
    )


def _band_tile(qg, kg):
    """[128, 256] bf16 band mask tile for global query rows qg, key cols kg."""
    kvalid = (kg >= 0) & (kg < SEQ)
    band = (np.abs(kg[None, :] - qg[:, None]) <= HALO) & kvalid[None, :]
    return band.astype(np.float32)


def prep_core_inputs(core, xs, pos, am, qkv_weight, out_weight, add_mask):
    """Build the per-core input map (numpy) for one core."""
    start = S_CORE * core - HALO
    idx = np.arange(start, start + SPAN)
    valid = (idx >= 0) & (idx < SEQ)

    Xs = np.zeros((HID, SPAN), np.float32)
    Xs[:, valid] = xs[:, idx[valid]]

    pspan = np.zeros((SPAN,), np.float32)
    pspan[valid] = pos[idx[valid]]
    invf = (
        1.0 / (10000.0 ** (np.arange(0, DH, 2, dtype=np.float32) / np.float32(DH)))
    ).astype(np.float32)
    f = pspan[None, :] * invf[:, None]  # [32, SPAN]
    COSb = np.tile(np.cos(f), (4, 1)).astype(ml_dtypes.bfloat16)
    SINb = np.tile(np.sin(f), (4, 1)).astype(ml_dtypes.bfloat16)

    # signed rotate-half permutation: (PERMS.T @ q)[d] = rot_half(q)[d]
    di = np.arange(128)
    lo = (di % 64) < 32
    src = np.where(lo, di + 32, di - 32)
    sgn = np.where(lo, -1.0, 1.0).astype(np.float32)
    PERMS = np.zeros((128, 128), np.float32)
    PERMS[src, di] = sgn

    # 3 mask slots: qb0 variant, interior, qb3 variant
    mb = np.zeros((128, 3, KSPAN), np.float32)
    for slot, qb in ((0, 0), (1, 1), (2, NQB - 1)):
        qg = S_CORE * core + 128 * qb + np.arange(128)
        kg = S_CORE * core + 128 * qb - HALO + np.arange(KSPAN)
        mb[:, slot] = _band_tile(qg, kg)

    # selector for broadcasting [8,128] reciprocal rows into [128, 512]:
    # BC[p, qb*128+c] = RR[2qb + (p>=64), c]
    sel = np.zeros((8, NQB * 128), np.float32)
    for qb in range(NQB):
        for half in range(2):
            sel[2 * qb + half, qb * 128 + 64 * half : qb * 128 + 64 * (half + 1)] = 1.0

    mf = None
    if add_mask:
        mf = np.full((128, NQB, 2, KSPAN), -10000.0, np.float32)
        for qb in range(NQB):
            qg = S_CORE * core + 128 * qb + np.arange(128)
            kg = S_CORE * core + 128 * qb - HALO + np.arange(KSPAN)
            kvalid = (kg >= 0) & (kg < SEQ)
            band = (np.abs(kg[None, :] - qg[:, None]) <= HALO) & kvalid[None, :]
            amband = np.zeros((128, KSPAN), np.float32)
            amband[:, kvalid] = am[np.ix_(qg, kg[kvalid])]
            m = np.where(band, amband, -10000.0)
            mf[:, qb, 0, :] = m
            mf[:, qb, 1, :] = m

    wq = qkv_weight[0:HID] * np.float32(DH**-0.5)
    wk = qkv_weight[HID : 2 * HID]
    wv = qkv_weight[2 * HID : 3 * HID]

    def packw(w):
        return _pack_chunked(
            np.ascontiguousarray(w.T.astype(ml_dtypes.bfloat16)), NCH, HID
        )

    def packw_v(w):
        # [c, o] -> [128, (hf, cchunk, 384)] so half-output slices are
        # contiguous (enables half-granularity prefetch)
        wt = np.ascontiguousarray(w.T.astype(ml_dtypes.bfloat16))  # [768c, 768o]
        a = wt.reshape(NCH, 128, 2, WV_HALF)  # (k, p, hf, j)
        return np.ascontiguousarray(
            a.transpose(1, 2, 0, 3).reshape(128, 2 * NCH * WV_HALF)
        )

    def packw_hp(w):
        # [c, o] -> [128, (hp, cchunk, 128)] so per-head-pair lhsT slices are
        # contiguous in the free dimension
        wt = np.ascontiguousarray(w.T.astype(ml_dtypes.bfloat16))  # [768c, 768o]
        a = wt.reshape(NCH, 128, NHP, 128)  # (cchunk, p, hp, n)
        return np.ascontiguousarray(
            a.transpose(1, 2, 0, 3).reshape(128, NHP * NCH * 128)
        )

    # q|k packed per head pair: [128, (hp, q 768 | k 768)]
    wqp = packw_hp(wq).reshape(128, NHP, NCH * 128)
    wkp = packw_hp(wk).reshape(128, NHP, NCH * 128)
    wqkt = np.ascontiguousarray(
        np.concatenate([wqp, wkp], axis=2).reshape(128, NHP * 2 * NCH * 128)
    )

    # misc pack: cos | sin | perms | mb | diag | sel (sel only rows 0-7)
    selp = np.zeros((128, NQB * 128), np.float32)
    selp[0:8] = sel
    misc = np.concatenate(
        [
            np.asarray(COSb, dtype=np.float32),
            np.asarray(SINb, dtype=np.float32),
            PERMS,
            mb.reshape(128, 3 * KSPAN),
            np.eye(128, dtype=np.float32),
            selp,
        ],
        axis=1,
    ).astype(ml_dtypes.bfloat16)

    in_map = {
        "xin": _pack_chunked(Xs.astype(ml_dtypes.bfloat16), NCH, SPAN),
        "wqkt": wqkt,
        "wvt": packw_v(wv),
        "wot": packw(out_weight),
        "miscb": np.ascontiguousarray(misc),
    }
    if add_mask:
        in_map["maskf"] = np.ascontiguousarray(mf.reshape(128, NQB * 2 * KSPAN))
    return in_map


def prep_all_inputs(x, position_ids, attention_mask, qkv_weight, out_weight):
    xs = np.asarray(x, dtype=np.float32)[0, :, 0, :]  # [768, 4096]
    pos = np.asarray(position_ids)[0].astype(np.float32)
    am = np.asarray(attention_mask, dtype=np.float32)[0, 0]
    qkv_w = np.asarray(qkv_weight, dtype=np.float32)
    out_w = np.asarray(out_weight, dtype=np.float32)
    add_mask = bool(np.any(am))
    in_maps = [
        prep_core_inputs(c, xs, pos, am, qkv_w, out_w, add_mask)
        for c in range(N_CORES)
    ]
    return in_maps, add_mask


def assemble_output(results):
    cols = []
    for c in range(N_CORES):
        o = np.asarray(results[c]["out"]).astype(np.float32)  # [128, 6*512] bf16
        cols.append(o.reshape(128, NCH, S_CORE).transpose(1, 0, 2).reshape(HID, S_CORE))
    full = np.concatenate(cols, axis=1)  # [768, 4096]
    return np.ascontiguousarray(full.reshape(1, HID, 1, SEQ), dtype=np.float32)


def kernel(**inputs):
    in_maps, add_mask = prep_all_inputs(
        inputs["x"],
        inputs["position_ids"],
        inputs["attention_mask"],
        inputs["qkv_weight"],
        inputs["out_weight"],
    )
    nc = get_program(add_mask)
    res = run_bass_kernel_spmd(nc, in_maps, core_ids=list(range(N_CORES)))
    return assemble_output(res.results)


# revision 98
# speedup vs baseline: 1.0282x; 1.0282x over previous
"""Trainium2 Bass kernel for sliding-window (±64) multi-head attention.

Reference computation (seq=4096, hidden=768, 12 heads x 64, RoPE, window 128):
    qkv = qkv_weight @ x ; q,k = rope(q,k) ; scores = q^T k / 8 + band_mask
    attn = softmax(scores) @ v ; out = out_weight @ attn

Sharding: sequence-parallel over 8 cores. Core c owns queries
[512c, 512c+512) and computes K/V over the haloed span [512c-64, 512c+576)
(zero-padded at the sequence edges; padding is killed by the band mask).
Each core runs an identical Bass program on different data; the full output
is reassembled on host by concatenation (no collectives needed).

Key structural choices (all bf16 matmuls; fp8 fails the 2e-2 rel-err gate):
- The HWDGE descriptor generator is one shared unit (~630ns per DMA) and
  the DMA wire is one shared serial FIFO, so inputs are packed into few
  large host-side tensors (wq|wk per head pair; cos|sin|perms|mask|diag|sel
  in one misc tensor) and DMA'd on the SP queue in strict first-use
  priority order; mid-run weights trickle between the P transposes.
- A PE p-state warm-up (throwaway matmuls on a zeroed tile) keeps the
  tensor engine busy while the first operands stream in, so the real
  projections start at the full 2.4GHz p-state.
- RoPE without evacuating the projection PSUM: since cos/sin are 32-periodic
  across each head's rotation pairs, rot(q*sin) == rot(q)*sin, so
  q_rope = q*cos + PERMS^T @ (q*sin).
- Softmax WITHOUT per-unit normalization on the critical path: the DVE
  scalar_tensor_tensor applies the multiplicative band mask and emits the
  row sums (accum_out) as a side effect; the un-normalized P goes straight
  to the transpose DMA and PV matmul. Normalization happens once per head
  pair at PSUM evacuation: row sums [128q, 8slots] -> bf16 -> PE transpose
  (identity-rhs matmul) -> bf16 reciprocal -> tiny selector matmuls
  broadcast the per-(h,q) reciprocals into a [128, 512] PSUM tile -> ACT
  copy to SBUF -> the evacuation is a single DVE tensor_tensor multiply
  (same cost as the copy it replaces; engines may read only one PSUM
  operand per instruction).
- P^T for the PV matmul comes from the DMA xbar transpose engine
  (dma_start_transpose): per unit one DMA does all four 128x128 block
  transposes (written packed at pitch 128 into a padded tile). The
  8-stage modulo pipeline gives the transpose ~3 steps of latency slack.
- Output projection: 5-chunk partials are computed while the last head
  pair's attention drains (oc 0-2 partials in f32, combined with the tail
  chunk on DVE; oc 3-5 in bf16, added back on PE via an identity matmul
  and evacuated on ACT, overlapping the DVE combines); the result ships
  in 2-chunk DMAs ordered by combine completion.
- Band mask is 3 shared [128,256] slots (first block / interior / last
  block); the output DMA is bf16 and upcast on host.
"""

import os
import sys

import numpy as np

for _p in ("/opt/trn_rl_repo",):
    if _p not in sys.path and os.path.isdir(_p):
        sys.path.insert(0, _p)

import ml_dtypes

import concourse.bass as bass
import concourse.bacc as bacc
import concourse.tile as tile
from concourse import mybir
from concourse.bass_utils import run_bass_kernel_spmd

F32 = mybir.dt.float32
BF16 = mybir.dt.bfloat16

N_CORES = 8
SEQ = 4096
S_CORE = SEQ // N_CORES  # 512 queries per core
HALO = 64                # window // 2
SPAN = S_CORE + 2 * HALO  # 640 keys per core
HID = 768
NH = 12
DH = 64
NCH = HID // 128         # 6 contraction chunks
NHP = NH // 2            # 6 head pairs
NQB = S_CORE // 128      # 4 query blocks per core
NSC = SPAN // 128        # 5 key chunks per core
KSPAN = 256              # key span per query block
PTS_PITCH = 136          # padded block pitch of the transposed-P tile
WV_HALF = HID // 2       # 384

_BUILD_CACHE = {}


def _build(add_mask: bool, reps: int = 1):
    """Build + compile the per-core Bass program (shared by all 8 cores)."""
    nc = bacc.Bacc("TRN2", target_bir_lowering=False, debug=False, num_devices=N_CORES)

    xin_d = nc.dram_tensor("xin", [128, NCH * SPAN], BF16, kind="ExternalInput")
    # q|k weights packed per head pair: [hp, (q 768 | k 768)]
    wqkt_d = nc.dram_tensor(
        "wqkt", [128, NHP * 2 * NCH * 128], BF16, kind="ExternalInput"
    )
    wvt_d = nc.dram_tensor("wvt", [128, NCH * HID], BF16, kind="ExternalInput")
    wot_d = nc.dram_tensor("wot", [128, NCH * HID], BF16, kind="ExternalInput")
    # misc pack: cos 640 | sin 640 | perms 128 | mb 768 | diag 128 | sel 512
    MISC_W = SPAN * 2 + 128 + 3 * KSPAN + 128 + NQB * 128
    misc_d = nc.dram_tensor("miscb", [128, MISC_W], BF16, kind="ExternalInput")
    if add_mask:
        maskf_d = nc.dram_tensor(
            "maskf", [128, NQB * 2 * KSPAN], F32, kind="ExternalInput"
        )
    out_d = nc.dram_tensor("out", [128, NCH * S_CORE], BF16, kind="ExternalOutput")

    mult = mybir.AluOpType.mult
    addop = mybir.AluOpType.add
    exp = mybir.ActivationFunctionType.Exp

    with tile.TileContext(nc) as tc:
        from contextlib import ExitStack

        for _rep in range(reps):
          with ExitStack() as ctx:
            const = ctx.enter_context(tc.tile_pool(name="const", bufs=1))
            sb = ctx.enter_context(tc.tile_pool(name="sb", bufs=1))
            rope_p = ctx.enter_context(tc.tile_pool(name="rope", bufs=10))
            attnp = ctx.enter_context(tc.tile_pool(name="attnp", bufs=6))
            scal = ctx.enter_context(tc.tile_pool(name="scal", bufs=12))
            outp = ctx.enter_context(tc.tile_pool(name="outp", bufs=6))
            ps_proj = ctx.enter_context(
                tc.tile_pool(name="ps_proj", bufs=3, space="PSUM")
            )
            ps_att = ctx.enter_context(
                tc.tile_pool(name="ps_att", bufs=3, space="PSUM")
            )
            ps_o = ctx.enter_context(tc.tile_pool(name="ps_o", bufs=2, space="PSUM"))

            # ---- input tiles ----
            XIN = const.tile([128, NCH * SPAN], BF16, tag="XIN")
            WQKT = const.tile([128, NHP * 2 * NCH * 128], BF16, tag="WQKT")
            WVT = const.tile([128, NCH * HID], BF16, tag="WVT")  # (hf, k, 384)
            MISC = const.tile([128, MISC_W], BF16, tag="MISC")
            COS = MISC[:, 0:SPAN]
            SIN = MISC[:, SPAN : 2 * SPAN]
            PERMS = MISC[:, 2 * SPAN : 2 * SPAN + 128]
            _mb0 = 2 * SPAN + 128
            MB = MISC[:, _mb0 : _mb0 + 3 * KSPAN]
            _dg0 = _mb0 + 3 * KSPAN
            DIAG = MISC[:, _dg0 : _dg0 + 128]
            _sl0 = _dg0 + 128
            SELB = MISC[0:8, _sl0 : _sl0 + NQB * 128]
            WOT = sb.tile([128, NCH * HID], BF16, tag="WOT")

            def dma_xin(k, eng):
                eng.dma_start(
                    out=XIN[:, k * SPAN : (k + 1) * SPAN],
                    in_=xin_d.ap()[:, k * SPAN : (k + 1) * SPAN],
                )

            def dma_wqk(hp, eng, half=None):
                w = 2 * NCH * 128
                lo, hi = hp * w, (hp + 1) * w
                if half == 0:
                    hi = lo + w // 2
                elif half == 1:
                    lo = lo + w // 2
                eng.dma_start(out=WQKT[:, lo:hi], in_=wqkt_d.ap()[:, lo:hi])

            def dma_wvt_half(hf, eng):
                w = NCH * WV_HALF
                eng.dma_start(
                    out=WVT[:, hf * w : (hf + 1) * w],
                    in_=wvt_d.ap()[:, hf * w : (hf + 1) * w],
                )

            # ---- startup DMA schedule. The HWDGE descriptor generator is a
            # single shared unit (~630ns per DMA) and the DMA wire is one
            # shared serial FIFO, so everything goes on the SP queue in
            # strict priority order of first use. Later bulk weights are
            # emitted inside the pipeline (extra schedule); since the SP SEQ
            # is held through each P-transpose's data wait, they naturally
            # trickle in without stealing early wire bandwidth.
            def dma_xin3(lo, eng):
                eng.dma_start(
                    out=XIN[:, lo * SPAN : (lo + 3) * SPAN],
                    in_=xin_d.ap()[:, lo * SPAN : (lo + 3) * SPAN],
                )

            # PE p-state warm-up: throwaway matmuls on a zeroed tile keep
            # the tensor engine continuously busy from t~0.2us until the
            # first real operands land (~4.3us), so the projection matmuls
            # start at the full 2.4GHz p-state instead of mid-ramp.
            WARM = sb.tile([128, 512], BF16, tag="WARM")
            nc.gpsimd.memset(WARM[:], 0.0)
            wps = ps_o.tile([128, 512], F32, tag="o", name="warm")
            for i in range(12):
                nc.tensor.matmul(wps[:], WARM[:, 0:128], WARM[:],
                                 start=(i == 0), stop=(i == 11))

            dma_wqk(0, nc.sync, half=0)   # wq of hp0
            dma_xin3(0, nc.sync)          # x chunks 0-2 (q proj can start)
            dma_xin3(3, nc.sync)          # x chunks 3-5
            dma_wqk(0, nc.sync, half=1)   # wk of hp0
            nc.sync.dma_start(   # cos | sin | perms
                out=MISC[:, 0 : 2 * SPAN + 128],
                in_=misc_d.ap()[:, 0 : 2 * SPAN + 128],
            )
            dma_wvt_half(0, nc.sync)
            dma_wqk(1, nc.sync)
            nc.sync.dma_start(   # mb | diag | sel
                out=MISC[:, 2 * SPAN + 128 : MISC_W],
                in_=misc_d.ap()[:, 2 * SPAN + 128 : MISC_W],
            )
            if add_mask:
                MF = const.tile([128, NQB * 2 * KSPAN], F32, tag="MF")
                nc.sync.dma_start(out=MF[:], in_=maskf_d.ap())

            # persistent intermediates
            Qs = sb.tile([128, NHP * S_CORE], BF16, tag="Qs")   # [2hd, (hp, s)]
            Ks = sb.tile([128, NHP * SPAN], BF16, tag="Ks")     # [2hd, (hp, s)]
            VT = sb.tile([128, NSC * HID], BF16, tag="VT")      # [s, (chunk, hd)]
            AT = sb.tile([128, NCH * S_CORE], BF16, tag="AT")   # [c, (cchunk, s)]

            def xs(k, lo, w):
                return XIN[:, k * SPAN + lo : k * SPAN + lo + w]

            # ---- V^T projection: VT[s, hd] per 128-key chunk (bf16).
            # Split by output half: half 0 feeds heads 0-5 (head pairs 0-2,
            # consumed from step 5), half 1 feeds heads 6-11 (from step 17),
            # so the second V-weight half can stream in late.
            def vt_unit(sc, hf):
                w = WV_HALF  # 384
                vp = ps_proj.tile([128, w], F32, tag="proj")
                for k in range(NCH):
                    nc.tensor.matmul(
                        vp[:],
                        xs(k, sc * 128, 128),
                        WVT[:, hf * NCH * w + k * w : hf * NCH * w + (k + 1) * w],
                        start=(k == 0),
                        stop=(k == NCH - 1),
                    )
                nc.scalar.copy(
                    VT[:, sc * HID + hf * w : sc * HID + (hf + 1) * w], vp[:]
                )

            # ---- per head pair: project Q,K then rope, in three phases so
            # the in-order PE queue never waits on elementwise results.
            rope_st = {}

            def proj_mm(hp):
                w2 = 2 * NCH * 128
                wq = WQKT[:, hp * w2 : hp * w2 + NCH * 128]
                wk = WQKT[:, hp * w2 + NCH * 128 : (hp + 1) * w2]
                blocks = []
                qp = ps_proj.tile([128, S_CORE], F32, tag="proj")
                for k in range(NCH):
                    nc.tensor.matmul(
                        qp[:],
                        wq[:, k * 128 : (k + 1) * 128],
                        xs(k, HALO, S_CORE),
                        start=(k == 0),
                        stop=(k == NCH - 1),
                    )
                blocks.append((qp, HALO, S_CORE,
                               Qs[:, hp * S_CORE : (hp + 1) * S_CORE]))
                for half in range(2):
                    w = SPAN // 2  # 320
                    kp = ps_proj.tile([128, w], F32, tag="proj")
                    for k in range(NCH):
                        nc.tensor.matmul(
                            kp[:],
                            wk[:, k * 128 : (k + 1) * 128],
                            xs(k, half * w, w),
                            start=(k == 0),
                            stop=(k == NCH - 1),
                        )
                    blocks.append(
                        (kp, half * w, w,
                         Ks[:, hp * SPAN + half * w : hp * SPAN + (half + 1) * w])
                    )
                rope_st[hp] = blocks

            def rope_mults(hp):
                out = []
                for i, (p, lo, w, dst) in enumerate(rope_st[hp]):
                    qsb = rope_p.tile([128, S_CORE], BF16, tag="qsb")
                    nc.scalar.copy(qsb[:, :w], p[:])
                    m1 = rope_p.tile([128, S_CORE], BF16, tag="m1")
                    m2 = rope_p.tile([128, S_CORE], BF16, tag="m2")
                    nc.gpsimd.tensor_tensor(
                        m1[:, :w], qsb[:, :w], COS[:, lo : lo + w], op=mult
                    )
                    nc.vector.tensor_tensor(
                        m2[:, :w], qsb[:, :w], SIN[:, lo : lo + w], op=mult
                    )
                    out.append((m1, m2, w, dst))
                rope_st[hp] = out

            def rope_rot(hp):
                for i, (m1, m2, w, dst) in enumerate(rope_st[hp]):
                    rot = ps_proj.tile([128, S_CORE], F32, tag="proj")
                    nc.tensor.matmul(
                        rot[:, :w], PERMS[:], m2[:, :w], start=True, stop=True
                    )
                    nc.vector.tensor_tensor(dst, m1[:, :w], rot[:, :w], op=addop)
                del rope_st[hp]

            # ---- attention: modulo software pipeline over the 24
            # (head-pair, query-block) units ----
            sss = {}   # hp -> [128, 8] f32 row-sum tile
            rrs = {}   # hp -> [8, 128] bf16 reciprocal tile
            o2s = {}

            # pair-granular pipeline: each unit is (hp, pb) covering query
            # blocks qb = 2pb, 2pb+1. Both query blocks of a head share one
            # PSUM bank (same PE row group -> sequential on the array, so
            # the shared bank is safe), exp runs [128,512] per head, and one
            # xbar DMA transposes the whole [128,1024] masked-P pair.
            def stage_scores(st):
                hp, pb = st["hp"], st["pb"]
                if pb == 0:
                    sss[hp] = scal.tile([128, 8], F32, tag="ss",
                                        name=f"ss_{hp}")
                sc = []
                for h in range(2):
                    s1 = ps_att.tile([128, 2 * KSPAN], F32, tag="att",
                                     name=f"s_{hp}_{pb}_{h}")
                    for u in range(2):
                        qb = 2 * pb + u
                        nc.tensor.matmul(
                            s1[:, u * KSPAN : (u + 1) * KSPAN],
                            Qs[64 * h : 64 * (h + 1),
                               hp * S_CORE + qb * 128 : hp * S_CORE + (qb + 1) * 128],
                            Ks[64 * h : 64 * (h + 1),
                               hp * SPAN + qb * 128 : hp * SPAN + qb * 128 + KSPAN],
                            start=True,
                            stop=True,
                        )
                    sc.append(s1)
                st["sc"] = sc

            def stage_exp(st):
                pb = st["pb"]
                praw = attnp.tile([128, 4 * KSPAN], BF16, tag="praw")
                for h in range(2):
                    sh = st["sc"][h][:]
                    if add_mask:
                        for u in range(2):
                            qb = 2 * pb + u
                            nc.vector.tensor_tensor(
                                st["sc"][h][:, u * KSPAN : (u + 1) * KSPAN],
                                st["sc"][h][:, u * KSPAN : (u + 1) * KSPAN],
                                MF[:, qb * 2 * KSPAN + h * KSPAN
                                   : qb * 2 * KSPAN + (h + 1) * KSPAN],
                                op=addop,
                            )
                    nc.scalar.activation(
                        praw[:, h * 2 * KSPAN : (h + 1) * 2 * KSPAN], sh, exp
                    )
                st["praw"] = praw
                del st["sc"]

            def stage_dve(st):
                hp, pb = st["hp"], st["pb"]
                praw = st["praw"]
                P = attnp.tile([128, 8 * 128], BF16, tag="P")
                ss = sss[hp]
                for u in range(2):
                    qb = 2 * pb + u
                    slot = 0 if qb == 0 else (2 if qb == NQB - 1 else 1)
                    for h in range(2):
                        b = u * 4 + 2 * h
                        nc.vector.scalar_tensor_tensor(
                            out=P[:, b * 128 : (b + 2) * 128],
                            in0=praw[:, h * 2 * KSPAN + u * KSPAN
                                     : h * 2 * KSPAN + (u + 1) * KSPAN],
                            scalar=1.0,
                            in1=MB[:, slot * KSPAN : (slot + 1) * KSPAN],
                            op0=mult,
                            op1=mult,
                            accum_out=ss[:, 2 * qb + h : 2 * qb + h + 1],
                        )
                st["P"] = P
                del st["praw"]

            def stage_pt(st):
                hp, pb = st["hp"], st["pb"]
                P = st["P"]
                ptsp = attnp.tile([128, 8 * PTS_PITCH], BF16, tag="pts")
                pts3 = ptsp[:].rearrange("p (b j) -> p b j", b=8)
                nc.sync.dma_start_transpose(out=pts3[:, :, 0:128], in_=P[:])
                st["pts"] = ptsp
                del st["P"]
                if pb == 1:
                    ssb = scal.tile([128, 8], BF16, tag="ssb")
                    nc.gpsimd.tensor_copy(ssb[:], sss[hp][:])
                    st["ssb"] = ssb

            def stage_bubble(st):
                hp, pb = st["hp"], st["pb"]
                if pb == 1:
                    sst = ps_proj.tile([8, 128], F32, tag="proj")
                    nc.tensor.matmul(
                        sst[:], st["ssb"][:], DIAG, start=True, stop=True
                    )
                    rr = scal.tile([8, 128], BF16, tag="rrb")
                    with nc.allow_low_precision("bf16 softmax scale is inside the rel-err budget"):
                        nc.vector.reciprocal(rr[:], sst[:])
                    rrs[hp] = rr
                    del sss[hp], st["ssb"]

            def stage_bubble2(st):
                hp, pb = st["hp"], st["pb"]
                if pb == 1:
                    bc = ps_o.tile([128, S_CORE], F32, tag="o",
                                   name=f"bc_{hp}")
                    rr = rrs[hp]
                    for qq in range(NQB):
                        nc.tensor.matmul(
                            bc[:, qq * 128 : (qq + 1) * 128],
                            SELB[:, qq * 128 : (qq + 1) * 128],
                            rr[:],
                            start=True, stop=True,
                        )
                    bcs = attnp.tile([128, S_CORE], BF16, tag="bcs")
                    nc.scalar.copy(bcs[:], bc[:])
                    st["bcs"] = bcs

            def stage_pv(st):
                hp, pb = st["hp"], st["pb"]
                if pb == 0:
                    o2s[hp] = ps_o.tile([128, S_CORE], F32, tag="o",
                                        name=f"o2_{hp}")
                o2 = o2s[hp]
                pts = st["pts"]
                for u in range(2):
                    qb = 2 * pb + u
                    for h in range(2):
                        hg = hp * 2 + h
                        osl = o2[64 * h : 64 * (h + 1),
                                 qb * 128 : (qb + 1) * 128]
                        tp = (0, 64 * h)
                        b = u * 4 + 2 * h
                        nc.tensor.matmul(
                            osl,
                            VT[:, qb * HID + hg * 64 : qb * HID + hg * 64 + 64],
                            pts[:, b * 128 : (b + 1) * 128],
                            start=True, stop=False, tile_position=tp,
                        )
                        nc.tensor.matmul(
                            osl,
                            VT[:, (qb + 1) * HID + hg * 64
                               : (qb + 1) * HID + hg * 64 + 64],
                            pts[:, (b + 1) * 128 : (b + 2) * 128],
                            start=False, stop=True, tile_position=tp,
                        )
                del st["pts"]
                if pb == 1:
                    nc.vector.tensor_tensor(
                        AT[:, hp * S_CORE : (hp + 1) * S_CORE], o2[:],
                        st["bcs"][:], op=mult,
                    )
                    del o2s[hp], rrs[hp], st["bcs"]

            # 5-chunk out-projection partials: oc 0-2 kept in f32 (combined
            # on DVE at the tail), oc 3-5 in bf16 (added back on PE via an
            # identity matmul, evacuated on ACT — overlaps the DVE combines)
            PO1 = sb.tile([128, 3 * S_CORE], F32, tag="PO1")
            PO1B = sb.tile([128, 3 * S_CORE], BF16, tag="PO1B")

            def outproj_part1(ocs):
                for oc in ocs:
                    ops = ps_proj.tile([128, S_CORE], F32, tag="proj")
                    for k in range(5):
                        nc.tensor.matmul(
                            ops[:],
                            WOT[:, k * HID + oc * 128 : k * HID + (oc + 1) * 128],
                            AT[:, k * S_CORE : (k + 1) * S_CORE],
                            start=(k == 0),
                            stop=(k == 4),
                        )
                    if oc in (1, 3, 5):
                        nc.scalar.copy(
                            PO1B[:, (oc // 2) * S_CORE : (oc // 2 + 1) * S_CORE],
                            ops[:],
                        )
                    else:
                        nc.scalar.copy(
                            PO1[:, (oc // 2) * S_CORE : (oc // 2 + 1) * S_CORE],
                            ops[:],
                        )

            proj_mm(0)
            rope_mults(0)
            vt_unit(0, 0)
            rope_rot(0)
            vt_unit(1, 0)

            units = [
                {"hp": hp, "pb": pb} for hp in range(NHP) for pb in range(2)
            ]
            stages = [stage_scores, stage_exp, stage_dve, stage_pt,
                      stage_bubble, stage_bubble2, stage_pv]
            NU = len(units)
            ND = len(stages)
            extra = {
                0: [lambda: proj_mm(1)],
                1: [lambda: rope_mults(1), lambda: rope_rot(1),
                    lambda: vt_unit(2, 0)],
                2: [lambda: vt_unit(3, 0)],
                3: [lambda: vt_unit(4, 0)],
                5: [lambda: vt_unit(0, 1)],
                6: [lambda: vt_unit(1, 1)],
                7: [lambda: vt_unit(2, 1)],
                8: [lambda: vt_unit(3, 1)],
                9: [lambda: vt_unit(4, 1)],
            }
            for hp_ in range(2, NHP):
                extra.setdefault(2 * hp_ - 3, []).append(
                    lambda h=hp_: proj_mm(h))
                extra.setdefault(2 * hp_ - 2, []).append(
                    lambda h=hp_: rope_mults(h))
                extra.setdefault(2 * hp_ - 1, []).append(
                    lambda h=hp_: rope_rot(h))
                extra.setdefault(max(0, 2 * hp_ - 5), []).append(
                    lambda h=hp_: dma_wqk(h, nc.sync))
            w_v = NCH * WV_HALF
            for i, st_ in enumerate((3, 4)):
                extra.setdefault(st_, []).append(
                    lambda i=i: nc.sync.dma_start(
                        out=WVT[:, w_v + i * w_v // 2 : w_v + (i + 1) * w_v // 2],
                        in_=wvt_d.ap()[:, w_v + i * w_v // 2 : w_v + (i + 1) * w_v // 2],
                    ))
            w_o = NCH * HID // 4
            for i, st_ in enumerate((4, 5, 6, 7)):
                extra.setdefault(st_, []).append(
                    lambda i=i: nc.sync.dma_start(
                        out=WOT[:, i * w_o : (i + 1) * w_o],
                        in_=wot_d.ap()[:, i * w_o : (i + 1) * w_o],
                    ))

            # AT chunk k lands at pv(hp=k, pb1) = step 2k+7
            part1_sched = {15: (0, 1), 16: (2, 3), 17: (4, 5)}

            for step in range(NU + ND - 1):
                for k in range(ND):
                    idx = step - k
                    if 0 <= idx < NU:
                        stages[k](units[idx])
                for fn in extra.get(step, ()):
                    fn()
                if step in part1_sched:
                    outproj_part1(part1_sched[step])

            # ---- output projection tail: last contraction chunk. oc 0-2
            # combine PSUM + PO1 on DVE; oc 3-5 add the bf16 partial back on
            # PE (identity-stationary matmul) and evacuate on ACT. Combines
            # write into one contiguous tile so each 3-chunk group ships in
            # a single DMA (HWDGE generation is serialized, ~630ns per DMA).
            OT = outp.tile([128, NCH * S_CORE], BF16, tag="ot")
            # each shipped pair gets one DVE combine (even oc) and one ACT
            # combine (odd oc) so the two engines drain the tail in parallel
            for oc in (1, 0, 3, 2, 5, 4):
                hold = oc in (1, 3, 5)
                pool = ps_att if hold else ps_proj
                ops = pool.tile([128, S_CORE], F32,
                                tag="att" if hold else "proj")
                nc.tensor.matmul(
                    ops[:],
                    WOT[:, 5 * HID + oc * 128 : 5 * HID + (oc + 1) * 128],
                    AT[:, 5 * S_CORE : 6 * S_CORE],
                    start=True,
                    stop=False if hold else True,
                )
                ot = OT[:, oc * S_CORE : (oc + 1) * S_CORE]
                if hold:
                    nc.tensor.matmul(
                        ops[:], DIAG,
                        PO1B[:, (oc // 2) * S_CORE : (oc // 2 + 1) * S_CORE],
                        start=False, stop=True,
                    )
                    nc.scalar.copy(ot, ops[:])
                else:
                    nc.vector.scalar_tensor_tensor(
                        out=ot, in0=ops[:], scalar=1.0,
                        in1=PO1[:, (oc // 2) * S_CORE : (oc // 2 + 1) * S_CORE],
                        op0=mult, op1=addop,
                    )
            # ship in 2-chunk pieces ordered by combine completion
            for pair, eng in (((0, 1), nc.sync), ((2, 3), nc.scalar),
                              ((4, 5), nc.sync)):
                lo = pair[0] * S_CORE
                hi = (pair[1] + 1) * S_CORE
                eng.dma_start(out=out_d.ap()[:, lo:hi], in_=OT[:, lo:hi])

    nc.compile()
    return nc


def get_program(add_mask: bool, reps: int = 1):
    key = (add_mask, reps)
    if key not in _BUILD_CACHE:
        _BUILD_CACHE[key] = _build(add_mask, reps)
    return _BUILD_CACHE[key]


def _pack_chunked(a, nch, w):
    """[nch*128, w] row-major -> [128, nch*w] with chunk-major free dim."""
    return np.ascontiguousarray(
        a.reshape(nch, 128, w).transpose(1, 0, 2).reshape(128, nch * w)
    )


def _band_tile(qg, kg):
    """[128, 256] bf16 band mask tile for global query rows qg, key cols kg."""
    kvalid = (kg >= 0) & (kg < SEQ)
    band = (np.abs(kg[None, :] - qg[:, None]) <= HALO) & kvalid[None, :]
    return band.astype(np.float32)


def prep_core_inputs(core, xs, pos, am, qkv_weight, out_weight, add_mask):
    """Build the per-core input map (numpy) for one core."""
    start = S_CORE * core - HALO
    idx = np.arange(start, start + SPAN)
    valid = (idx >= 0) & (idx < SEQ)

    Xs = np.zeros((HID, SPAN), np.float32)
    Xs[:, valid] = xs[:, idx[valid]]

    pspan = np.zeros((SPAN,), np.float32)
    pspan[valid] = pos[idx[valid]]
    invf = (
        1.0 / (10000.0 ** (np.arange(0, DH, 2, dtype=np.float32) / np.float32(DH)))
    ).astype(np.float32)
    f = pspan[None, :] * invf[:, None]  # [32, SPAN]
    COSb = np.tile(np.cos(f), (4, 1)).astype(ml_dtypes.bfloat16)
    SINb = np.tile(np.sin(f), (4, 1)).astype(ml_dtypes.bfloat16)

    # signed rotate-half permutation: (PERMS.T @ q)[d] = rot_half(q)[d]
    di = np.arange(128)
    lo = (di % 64) < 32
    src = np.where(lo, di + 32, di - 32)
    sgn = np.where(lo, -1.0, 1.0).astype(np.float32)
    PERMS = np.zeros((128, 128), np.float32)
    PERMS[src, di] = sgn

    # 3 mask slots: qb0 variant, interior, qb3 variant
    mb = np.zeros((128, 3, KSPAN), np.float32)
    for slot, qb in ((0, 0), (1, 1), (2, NQB - 1)):
        qg = S_CORE * core + 128 * qb + np.arange(128)
        kg = S_CORE * core + 128 * qb - HALO + np.arange(KSPAN)
        mb[:, slot] = _band_tile(qg, kg)

    # selector for broadcasting [8,128] reciprocal rows into [128, 512]:
    # BC[p, qb*128+c] = RR[2qb + (p>=64), c]
    sel = np.zeros((8, NQB * 128), np.float32)
    for qb in range(NQB):
        for half in range(2):
            sel[2 * qb + half, qb * 128 + 64 * half : qb * 128 + 64 * (half + 1)] = 1.0

    mf = None
    if add_mask:
        mf = np.full((128, NQB, 2, KSPAN), -10000.0, np.float32)
        for qb in range(NQB):
            qg = S_CORE * core + 128 * qb + np.arange(128)
            kg = S_CORE * core + 128 * qb - HALO + np.arange(KSPAN)
            kvalid = (kg >= 0) & (kg < SEQ)
            band = (np.abs(kg[None, :] - qg[:, None]) <= HALO) & kvalid[None, :]
            amband = np.zeros((128, KSPAN), np.float32)
            amband[:, kvalid] = am[np.ix_(qg, kg[kvalid])]
            m = np.where(band, amband, -10000.0)
            mf[:, qb, 0, :] = m
            mf[:, qb, 1, :] = m

    wq = qkv_weight[0:HID] * np.float32(DH**-0.5)
    wk = qkv_weight[HID : 2 * HID]
    wv = qkv_weight[2 * HID : 3 * HID]

    def packw(w):
        return _pack_chunked(
            np.ascontiguousarray(w.T.astype(ml_dtypes.bfloat16)), NCH, HID
        )

    def packw_v(w):
        # [c, o] -> [128, (hf, cchunk, 384)] so half-output slices are
        # contiguous (enables half-granularity prefetch)
        wt = np.ascontiguousarray(w.T.astype(ml_dtypes.bfloat16))  # [768c, 768o]
        a = wt.reshape(NCH, 128, 2, WV_HALF)  # (k, p, hf, j)
        return np.ascontiguousarray(
            a.transpose(1, 2, 0, 3).reshape(128, 2 * NCH * WV_HALF)
        )

    def packw_hp(w):
        # [c, o] -> [128, (hp, cchunk, 128)] so per-head-pair lhsT slices are
        # contiguous in the free dimension
        wt = np.ascontiguousarray(w.T.astype(ml_dtypes.bfloat16))  # [768c, 768o]
        a = wt.reshape(NCH, 128, NHP, 128)  # (cchunk, p, hp, n)
        return np.ascontiguousarray(
            a.transpose(1, 2, 0, 3).reshape(128, NHP * NCH * 128)
        )

    # q|k packed per head pair: [128, (hp, q 768 | k 768)]
    wqp = packw_hp(wq).reshape(128, NHP, NCH * 128)
    wkp = packw_hp(wk).reshape(128, NHP, NCH * 128)
    wqkt = np.ascontiguousarray(
        np.concatenate([wqp, wkp], axis=2).reshape(128, NHP * 2 * NCH * 128)
    )

    # misc pack: cos | sin | perms | mb | diag | sel (sel only rows 0-7)
    selp = np.zeros((128, NQB * 128), np.float32)
    selp[0:8] = sel
    misc = np.concatenate(
        [
            np.asarray(COSb, dtype=np.float32),
            np.asarray(SINb, dtype=np.float32),
            PERMS,
            mb.reshape(128, 3 * KSPAN),
            np.eye(128, dtype=np.float32),
            selp,
        ],
        axis=1,
    ).astype(ml_dtypes.bfloat16)

    in_map = {
        "xin": _pack_chunked(Xs.astype(ml_dtypes.bfloat16), NCH, SPAN),
        "wqkt": wqkt,
        "wvt": packw_v(wv),
        "wot": packw(out_weight),
        "miscb": np.ascontiguousarray(misc),
    }
    if add_mask:
        in_map["maskf"] = np.ascontiguousarray(mf.reshape(128, NQB * 2 * KSPAN))
    return in_map


def prep_all_inputs(x, position_ids, attention_mask, qkv_weight, out_weight):
    xs = np.asarray(x, dtype=np.float32)[0, :, 0, :]  # [768, 4096]
    pos = np.asarray(position_ids)[0].astype(np.float32)
    am = np.asarray(attention_mask, dtype=np.float32)[0, 0]
    qkv_w = np.asarray(qkv_weight, dtype=np.float32)
    out_w = np.asarray(out_weight, dtype=np.float32)
    add_mask = bool(np.any(am))
    in_maps = [
        prep_core_inputs(c, xs, pos, am, qkv_w, out_w, add_mask)
        for c in range(N_CORES)
    ]
    return in_maps, add_mask


def assemble_output(results):
    cols = []
    for c in range(N_CORES):
        o = np.asarray(results[c]["out"]).astype(np.float32)  # [128, 6*512] bf16
        cols.append(o.reshape(128, NCH, S_CORE).transpose(1, 0, 2).reshape(HID, S_CORE))
    full = np.concatenate(cols, axis=1)  # [768, 4096]
    return np.ascontiguousarray(full.reshape(1, HID, 1, SEQ), dtype=np.float32)


def kernel(**inputs):
    in_maps, add_mask = prep_all_inputs(
        inputs["x"],
        inputs["position_ids"],
        inputs["attention_mask"],
        inputs["qkv_weight"],
        inputs["out_weight"],
    )
    nc = get_program(add_mask)
    res = run_bass_kernel_spmd(nc, in_maps, core_ids=list(range(N_CORES)))
    return assemble_output(res.results)
